# revision 22
# baseline (speedup 1.0000x reference)
"""GQA causal attention with rope, 8-way head-tensor-parallel on one TRN2 chip.

Strategy (per core c of 8): q-heads 4c..4c+3, kv-head c.
  - Host pre-transposes/pre-tiles x, permutes wq/wk rows for cheap on-device rope,
    duplicates the k rows (k2) so k^T lands duplicated across both partition
    halves for row-tiled score matmuls, casts weights+x to bf16.
  - Device: QKV projection (TensorE, bf16) -> rope (VectorE, free-dim pair blocks)
    -> DMA-transposes to q^T/k^T layout -> scores S^T = K Q^T (row-tiled 64x128
    matmul pairs, 2 heads concurrently, into one 2-bank PSUM tile) -> single
    paired exp on ScalarE (no max subtraction; scores ~ N(0,1)) -> causal mask
    via gpsimd affine_select on diagonal tiles -> P^T V with a fused ones-column
    producing softmax denominators -> normalize via partition_broadcast +
    ln/exp reciprocal -> output projection interleaved per qs-chunk.
  - Each core DMAs out its partial out^T [B, D, S]; host sums partials over
    cores and transposes back.
"""
import sys
for _p in ("/opt/trn_rl_repo",):
    if _p not in sys.path:
        sys.path.insert(0, _p)

import numpy as np
import ml_dtypes

B, S, DIM = 2, 2048, 2048
NH, NKV, HD = 32, 8, 64
P = 128
ST = S // P          # 16 s-tiles
CT = DIM // P        # 16 contraction tiles
NCORE = 8
HPC = NH // NCORE    # 4 q heads per core
QKV = 384            # 4*64 q + 64 k + 64 v columns
NROPE = 320          # rope'd columns (q + k)
NCH = 4              # qs chunks of 512
CHW = 512

_nc_cache = None


def build_nc():
    import concourse.bass as bass
    import concourse.mybir as mybir
    import concourse.tile as tile
    from concourse import bacc
    from concourse.masks import make_identity

    f32 = mybir.dt.float32
    bf16 = mybir.dt.bfloat16

    nc = bacc.Bacc("TRN2", target_bir_lowering=False)
    xt_d = nc.declare_dram_parameter("xt", [B, ST, P, CT, P], bf16, isOutput=False)
    w_d = nc.declare_dram_parameter("wqkv", [P, CT, QKV], bf16, isOutput=False)
    wo_d = nc.declare_dram_parameter("wo", [P, 2, DIM], bf16, isOutput=False)
    fc_d = nc.declare_dram_parameter("fcos", [P, ST, 32], f32, isOutput=False)
    fs_d = nc.declare_dram_parameter("fsin", [P, ST, 32], f32, isOutput=False)
    out_d = nc.declare_dram_parameter("out", [B, ST, P, NCH, CHW], f32, isOutput=True)

    AP = bass.AP

    def blocks(t, col0, nblk, bstride=64):
        """AP over `nblk` 32-wide col blocks of 2D tile t starting at col0, stride bstride."""
        a = t if isinstance(t, AP) else t[:]
        return AP(tensor=a.tensor, offset=a.offset + col0, ap=[a.ap[0], [bstride, nblk], [1, 32]])

    def bcast32(a, nblk):
        """Broadcast a [128, 32] AP across nblk col blocks."""
        return AP(tensor=a.tensor, offset=a.offset, ap=[a.ap[0], [0, nblk], [1, 32]])

    with tile.TileContext(nc) as tc:
        with (
            tc.tile_pool(name="const", bufs=1) as cst,
            tc.tile_pool(name="work", bufs=3) as work,
            tc.tile_pool(name="perb", bufs=2) as perb,
            tc.tile_pool(name="pp", bufs=18) as pp,
            tc.tile_pool(name="norm", bufs=3) as norm,
            tc.tile_pool(name="normu", bufs=6) as normu,
            tc.tile_pool(name="outp", bufs=4) as outp,
            tc.tile_pool(name="ps_sc", bufs=3, space="PSUM") as ps_sc,
            tc.tile_pool(name="ps_u", bufs=2, space="PSUM") as ps_u,
        ):
            w_sb = cst.tile([P, CT, QKV], bf16, tag="w")
            nc.sync.dma_start(out=w_sb[:], in_=w_d[:])
            wo_sb = cst.tile([P, 2, DIM], bf16, tag="wo")
            nc.sync.dma_start(out=wo_sb[:], in_=wo_d[:])
            fc_sb = cst.tile([P, ST, 32], f32, tag="fc")
            nc.sync.dma_start(out=fc_sb[:], in_=fc_d[:])
            fs_sb = cst.tile([P, ST, 32], f32, tag="fs")
            nc.sync.dma_start(out=fs_sb[:], in_=fs_d[:])
            ident = cst.tile([P, P], bf16, tag="id")
            make_identity(nc, ident)

            for b in range(B):
                qt01 = perb.tile([P, S], bf16, tag="qt01")
                qt23 = perb.tile([P, S], bf16, tag="qt23")
                ktd = perb.tile([P, S], bf16, tag="ktd")
                v1 = perb.tile([P, ST, 65], bf16, tag="v1")
                ao01 = perb.tile([P, S], bf16, tag="ao01")
                ao23 = perb.tile([P, S], bf16, tag="ao23")
                nc.vector.memset(v1[:], 1.0)  # ones col; data cols overwritten below

                # ---- Phase A: QKV projection + rope + transposes ----
                for st in range(ST):
                    xt = work.tile([P, CT, P], bf16, tag="xt")
                    nc.sync.dma_start(out=xt[:], in_=xt_d[b, st])
                    pmm = ps_sc.tile([P, 2, CHW], f32, tag="sc")
                    for ct in range(CT):
                        nc.tensor.matmul(
                            pmm[:, 0, 0:QKV], lhsT=xt[:, ct, :], rhs=w_sb[:, ct, :],
                            start=(ct == 0), stop=(ct == CT - 1),
                        )
                    pm = pmm[:, 0, 0:QKV]
                    cos_st = fc_sb[:, st, :]
                    sin_st = fs_sb[:, st, :]
                    tA = work.tile([P, NROPE], f32, tag="tA")
                    tB = work.tile([P, NROPE], f32, tag="tB")
                    # tA = pm * cos on all 12 rope blocks (q0..q3,k,k2) x (t0,t1)
                    nc.vector.tensor_mul(blocks(tA, 0, 10, 32), blocks(pm, 0, 10, 32), bcast32(cos_st, 10))
                    # tB[t0 blocks] = pm[t1 blocks] * sin ; tB[t1] = pm[t0] * sin
                    nc.vector.tensor_mul(blocks(tB, 0, 5), blocks(pm, 32, 5), bcast32(sin_st, 5))
                    nc.vector.tensor_mul(blocks(tB, 32, 5), blocks(pm, 0, 5), bcast32(sin_st, 5))
                    qk = work.tile([P, NROPE], bf16, tag="qk")
                    nc.vector.tensor_sub(blocks(qk, 0, 5), blocks(tA, 0, 5), blocks(tB, 0, 5))
                    nc.vector.tensor_add(blocks(qk, 32, 5), blocks(tA, 32, 5), blocks(tB, 32, 5))
                    nc.vector.tensor_copy(v1[:, st, 0:64], pm[:, NROPE:QKV])
                    # PE transposes: q01, q23 [128,128]; k [128,64] dup'd to both halves
                    for j, dest in ((0, qt01), (1, qt23)):
                        ptr = ps_u.tile([P, CHW], mybir.dt.bfloat16, tag="u")
                        nc.tensor.transpose(ptr[:, 0:P], qk[:, j * P:(j + 1) * P], ident[:])
                        if j == 0:
                            nc.scalar.copy(out=dest[:, st * P:(st + 1) * P], in_=ptr[:, 0:P])
                        else:
                            nc.vector.tensor_copy(dest[:, st * P:(st + 1) * P], ptr[:, 0:P])
                    ptk = ps_u.tile([P, CHW], mybir.dt.bfloat16, tag="u")
                    nc.tensor.transpose(ptk[0:64, 0:P], qk[:, 256:320], ident[:])
                    nc.scalar.copy(out=ktd[0:64, st * P:(st + 1) * P], in_=ptk[0:64, 0:P])
                    nc.vector.tensor_copy(ktd[64:128, st * P:(st + 1) * P], ptk[0:64, 0:P])

                # ---- Phases B+C interleaved: O-proj pipelined one chunk behind ----
                def oproj_chunk(ch):
                    for dot in range(ST):
                        po = ps_u.tile([P, CHW], f32, tag="u")
                        nc.tensor.matmul(po[:], lhsT=wo_sb[:, 0, dot * P:(dot + 1) * P],
                                         rhs=ao01[:, ch * CHW:(ch + 1) * CHW], start=True, stop=False)
                        nc.tensor.matmul(po[:], lhsT=wo_sb[:, 1, dot * P:(dot + 1) * P],
                                         rhs=ao23[:, ch * CHW:(ch + 1) * CHW], start=False, stop=True)
                        so = outp.tile([P, CHW], f32, tag="so")
                        if dot % 2 == 0:
                            nc.scalar.copy(out=so[:], in_=po[:])
                        else:
                            nc.vector.tensor_copy(so[:], po[:])
                        nc.sync.dma_start(out=out_d[b, dot, :, ch, :], in_=so[:])

                prev_ch = None
                for ch in (3, 2, 1, 0):
                    nks = 4 * (ch + 1)
                    for pair, (qt, ao) in enumerate(((qt01, ao01), (qt23, ao23))):
                        ppr = []
                        for kst in range(nks):
                            psc = ps_sc.tile([P, 2, CHW], f32, tag="sc")
                            nc.tensor.matmul(
                                psc[:, 0, :], lhsT=ktd[0:64, kst * P:(kst + 1) * P],
                                rhs=qt[0:64, ch * CHW:(ch + 1) * CHW], start=True, stop=True)
                            nc.tensor.matmul(
                                psc[:, 1, :], lhsT=ktd[64:128, kst * P:(kst + 1) * P],
                                rhs=qt[64:128, ch * CHW:(ch + 1) * CHW], start=True, stop=True)
                            pt = pp.tile([P, 2, CHW], mybir.dt.bfloat16, tag="p")
                            nc.scalar.activation(pt[:], psc[:], mybir.ActivationFunctionType.Exp, scale=0.125)
                            o = kst - 4 * ch
                            if o >= 0:
                                nc.gpsimd.affine_select(
                                    out=pt[:], in_=pt[:],
                                    compare_op=mybir.AluOpType.is_ge,
                                    fill=0.0, base=-P * o, channel_multiplier=-1,
                                    pattern=[[0, 2], [1, CHW]],
                                )
                            ppr.append(pt)
                        u0 = ps_u.tile([P, CHW], f32, tag="u")
                        u1 = ps_u.tile([P, CHW], f32, tag="u")
                        for kst in range(nks):
                            nc.tensor.matmul(u0[0:65, :], lhsT=v1[:, kst, :], rhs=ppr[kst][:, 0, :],
                                             start=(kst == 0), stop=(kst == nks - 1))
                            nc.tensor.matmul(u1[0:65, :], lhsT=v1[:, kst, :], rhs=ppr[kst][:, 1, :],
                                             start=(kst == 0), stop=(kst == nks - 1))
                        for u, basep in ((u0, 0), (u1, 64)):
                            ao_sl = ao[basep:basep + 64, ch * CHW:(ch + 1) * CHW]
                            nc.vector.tensor_copy(ao_sl, u[0:64, :])
                            dns = norm.tile([1, CHW], f32, tag="dns")
                            nc.vector.tensor_copy(dns[:], u[64:65, :])
                            dn = norm.tile([1, CHW], f32, tag="dn")
                            nc.vector.reciprocal(dn[:], dns[:])
                            bcs = normu.tile([P, CHW], f32, tag="bcs")
                            nc.gpsimd.partition_broadcast(bcs[:], dn[:])
                            if basep == 0:
                                nc.gpsimd.tensor_mul(ao_sl, ao_sl, bcs[0:64, :])
                            else:
                                nc.vector.tensor_mul(ao_sl, ao_sl, bcs[64:128, :])

                    # O-projection of the previous chunk (deps long satisfied)
                    if prev_ch is not None:
                        oproj_chunk(prev_ch)
                    prev_ch = ch
                oproj_chunk(prev_ch)

    nc.compile()
    return nc


def get_nc():
    global _nc_cache
    if _nc_cache is None:
        _nc_cache = build_nc()
    return _nc_cache


def prep_inputs(x, freqs_cos, freqs_sin, wq, wk, wv, wo):
    """Host-side layout prep. Returns list of per-core input dicts."""
    bf = ml_dtypes.bfloat16
    x = np.asarray(x, dtype=np.float32)
    # xh[b, st, p, ct, sl] = x[b, st*128+sl, ct*128+p]
    xh = np.ascontiguousarray(
        x.reshape(B, ST, P, CT, P).transpose(0, 1, 4, 3, 2).astype(bf))
    # fc[p, st, j] = freqs_cos[st*128+p, j]
    fc = np.ascontiguousarray(
        np.asarray(freqs_cos, np.float32).reshape(ST, P, 32).transpose(1, 0, 2))
    fs = np.ascontiguousarray(
        np.asarray(freqs_sin, np.float32).reshape(ST, P, 32).transpose(1, 0, 2))
    perm = np.concatenate([np.arange(0, HD, 2), np.arange(1, HD, 2)])
    in_maps = []
    for c in range(NCORE):
        q_rows = np.asarray(wq, np.float32)[c * HPC * HD:(c + 1) * HPC * HD]
        q_rows = q_rows.reshape(HPC, HD, DIM)[:, perm, :].reshape(HPC * HD, DIM)
        k_rows = np.asarray(wk, np.float32)[c * HD:(c + 1) * HD][perm]
        v_rows = np.asarray(wv, np.float32)[c * HD:(c + 1) * HD]
        wcat = np.concatenate([q_rows, k_rows, v_rows], axis=0)  # [384, DIM]
        w_h = np.ascontiguousarray(wcat.T.reshape(CT, P, QKV).transpose(1, 0, 2).astype(bf))
        wo_cols = np.asarray(wo, np.float32)[:, c * HPC * HD:(c + 1) * HPC * HD]  # [DIM, 256]
        wo_h = np.ascontiguousarray(wo_cols.T.reshape(2, P, DIM).transpose(1, 0, 2).astype(bf))
        in_maps.append({"xt": xh, "wqkv": w_h, "wo": wo_h, "fcos": fc, "fsin": fs})
    return in_maps


def combine_outputs(results):
    """Sum per-core partial out^T and return [B, S, DIM] float32."""
    acc = np.zeros((B, ST, P, NCH, CHW), np.float64)
    for r in results:
        acc += r["out"].astype(np.float64)
    # out[b, ch*512+sl, dot*128+p] = acc[b, dot, p, ch, sl]
    return np.ascontiguousarray(
        acc.transpose(0, 3, 4, 1, 2).reshape(B, S, DIM).astype(np.float32))


def kernel(x, freqs_cos, freqs_sin, wq, wk, wv, wo):
    from concourse.bass_utils import run_bass_kernel_spmd

    nc = get_nc()
    in_maps = prep_inputs(x, freqs_cos, freqs_sin, wq, wk, wv, wo)
    res = run_bass_kernel_spmd(nc, in_maps, core_ids=list(range(NCORE)))
    return combine_outputs(res.results)


# revision 23
# speedup vs baseline: 1.4266x; 1.4266x over previous
"""GQA causal attention with rope, 8-way head-tensor-parallel on one TRN2 chip.

Strategy (per core c of 8): q-heads 4c..4c+3, kv-head c.
  - Host pre-transposes/pre-tiles x, permutes wq/wk rows for cheap on-device rope,
    duplicates the k rows (k2) so k^T lands duplicated across both partition
    halves for row-tiled score matmuls, casts weights+x to bf16.
  - Device: QKV projection (TensorE, bf16) -> rope (VectorE, free-dim pair blocks)
    -> DMA-transposes to q^T/k^T layout -> scores S^T = K Q^T (row-tiled 64x128
    matmul pairs, 2 heads concurrently, into one 2-bank PSUM tile) -> single
    paired exp on ScalarE (no max subtraction; scores ~ N(0,1)) -> causal mask
    via gpsimd affine_select on diagonal tiles -> P^T V with a fused ones-column
    producing softmax denominators -> normalize via partition_broadcast +
    ln/exp reciprocal -> output projection interleaved per qs-chunk.
  - Each core DMAs out its partial out^T [B, D, S]; host sums partials over
    cores and transposes back.
"""
import sys
for _p in ("/opt/trn_rl_repo",):
    if _p not in sys.path:
        sys.path.insert(0, _p)

import numpy as np
import ml_dtypes

B, S, DIM = 2, 2048, 2048
NH, NKV, HD = 32, 8, 64
P = 128
ST = S // P          # 16 s-tiles
CT = DIM // P        # 16 contraction tiles
NCORE = 8
HPC = NH // NCORE    # 4 q heads per core
QKV = 384            # 4*64 q + 64 k + 64 v columns
NROPE = 320          # rope'd columns (q + k)
NCH = 4              # qs chunks of 512
CHW = 512

_nc_cache = None


def build_nc():
    import concourse.bass as bass
    import concourse.mybir as mybir
    import concourse.tile as tile
    from concourse import bacc
    from concourse.masks import make_identity

    f32 = mybir.dt.float32
    bf16 = mybir.dt.bfloat16

    nc = bacc.Bacc("TRN2", target_bir_lowering=False)
    xt_d = nc.declare_dram_parameter("xt", [B, ST, P, CT, P], bf16, isOutput=False)
    w_d = nc.declare_dram_parameter("wqkv", [P, CT, QKV], bf16, isOutput=False)
    wo_d = nc.declare_dram_parameter("wo", [P, 2, DIM], bf16, isOutput=False)
    fc_d = nc.declare_dram_parameter("fcos", [P, ST, 32], f32, isOutput=False)
    fs_d = nc.declare_dram_parameter("fsin", [P, ST, 32], f32, isOutput=False)
    out_d = nc.declare_dram_parameter("out", [B, ST, P, NCH, CHW], f32, isOutput=True)

    AP = bass.AP

    def blocks(t, col0, nblk, bstride=64):
        """AP over `nblk` 32-wide col blocks of 2D tile t starting at col0, stride bstride."""
        a = t if isinstance(t, AP) else t[:]
        return AP(tensor=a.tensor, offset=a.offset + col0, ap=[a.ap[0], [bstride, nblk], [1, 32]])

    def bcast32(a, nblk):
        """Broadcast a [128, 32] AP across nblk col blocks."""
        return AP(tensor=a.tensor, offset=a.offset, ap=[a.ap[0], [0, nblk], [1, 32]])

    with tile.TileContext(nc) as tc:
        with (
            tc.tile_pool(name="const", bufs=1) as cst,
            tc.tile_pool(name="work", bufs=3) as work,
            tc.tile_pool(name="perb", bufs=2) as perb,
            tc.tile_pool(name="pp", bufs=18) as pp,
            tc.tile_pool(name="norm", bufs=3) as norm,
            tc.tile_pool(name="normu", bufs=6) as normu,
            tc.tile_pool(name="outp", bufs=4) as outp,
            tc.tile_pool(name="ps_sc", bufs=3, space="PSUM") as ps_sc,
            tc.tile_pool(name="ps_u", bufs=2, space="PSUM") as ps_u,
        ):
            w_sb = cst.tile([P, CT, QKV], bf16, tag="w")
            nc.sync.dma_start(out=w_sb[:], in_=w_d[:])
            wo_sb = cst.tile([P, 2, DIM], bf16, tag="wo")
            nc.sync.dma_start(out=wo_sb[:], in_=wo_d[:])
            fc_sb = cst.tile([P, ST, 32], f32, tag="fc")
            nc.sync.dma_start(out=fc_sb[:], in_=fc_d[:])
            fs_sb = cst.tile([P, ST, 32], f32, tag="fs")
            nc.sync.dma_start(out=fs_sb[:], in_=fs_d[:])
            ident = cst.tile([P, P], bf16, tag="id")
            make_identity(nc, ident)

            for b in range(B):
                qt01 = perb.tile([P, S], bf16, tag="qt01")
                qt23 = perb.tile([P, S], bf16, tag="qt23")
                ktd = perb.tile([P, S], bf16, tag="ktd")
                v1 = perb.tile([P, ST, 65], bf16, tag="v1")
                ao01 = perb.tile([P, S], bf16, tag="ao01")
                ao23 = perb.tile([P, S], bf16, tag="ao23")
                nc.vector.memset(v1[:], 1.0)  # ones col; data cols overwritten below

                # ---- Phase A: QKV projection + rope + transposes ----
                for st in range(ST):
                    xt = work.tile([P, CT, P], bf16, tag="xt")
                    nc.sync.dma_start(out=xt[:], in_=xt_d[b, st])
                    pmm = ps_sc.tile([P, 2, CHW], f32, tag="sc")
                    for ct in range(CT):
                        nc.tensor.matmul(
                            pmm[:, 0, 0:QKV], lhsT=xt[:, ct, :], rhs=w_sb[:, ct, :],
                            start=(ct == 0), stop=(ct == CT - 1),
                        )
                    pm = pmm[:, 0, 0:QKV]
                    cos_st = fc_sb[:, st, :]
                    sin_st = fs_sb[:, st, :]
                    tA = work.tile([P, NROPE], f32, tag="tA")
                    tB = work.tile([P, NROPE], f32, tag="tB")
                    # tA = pm * cos on all 12 rope blocks (q0..q3,k,k2) x (t0,t1)
                    nc.vector.tensor_mul(blocks(tA, 0, 10, 32), blocks(pm, 0, 10, 32), bcast32(cos_st, 10))
                    # tB[t0 blocks] = pm[t1 blocks] * sin ; tB[t1] = pm[t0] * sin
                    nc.vector.tensor_mul(blocks(tB, 0, 5), blocks(pm, 32, 5), bcast32(sin_st, 5))
                    nc.vector.tensor_mul(blocks(tB, 32, 5), blocks(pm, 0, 5), bcast32(sin_st, 5))
                    qk = work.tile([P, NROPE], bf16, tag="qk")
                    nc.vector.tensor_sub(blocks(qk, 0, 5), blocks(tA, 0, 5), blocks(tB, 0, 5))
                    nc.vector.tensor_add(blocks(qk, 32, 5), blocks(tA, 32, 5), blocks(tB, 32, 5))
                    nc.vector.tensor_copy(v1[:, st, 0:64], pm[:, NROPE:QKV])
                    # PE transposes: q01, q23 [128,128]; k [128,64] dup'd to both halves
                    for j, dest in ((0, qt01), (1, qt23)):
                        ptr = ps_u.tile([P, CHW], mybir.dt.bfloat16, tag="u")
                        nc.tensor.transpose(ptr[:, 0:P], qk[:, j * P:(j + 1) * P], ident[:])
                        if j == 0:
                            nc.scalar.copy(out=dest[:, st * P:(st + 1) * P], in_=ptr[:, 0:P])
                        else:
                            nc.vector.tensor_copy(dest[:, st * P:(st + 1) * P], ptr[:, 0:P])
                    ptk = ps_u.tile([P, CHW], mybir.dt.bfloat16, tag="u")
                    nc.tensor.transpose(ptk[0:64, 0:P], qk[:, 256:320], ident[:])
                    nc.scalar.copy(out=ktd[0:64, st * P:(st + 1) * P], in_=ptk[0:64, 0:P])
                    nc.vector.tensor_copy(ktd[64:128, st * P:(st + 1) * P], ptk[0:64, 0:P])

                # ---- Phases B+C interleaved: O-proj pipelined one chunk behind ----
                def oproj_chunk(ch):
                    for dot in range(ST):
                        po = ps_sc.tile([P, 2, CHW], f32, tag="sc")
                        nc.tensor.matmul(po[:, 0, :], lhsT=wo_sb[:, 0, dot * P:(dot + 1) * P],
                                         rhs=ao01[:, ch * CHW:(ch + 1) * CHW], start=True, stop=False)
                        nc.tensor.matmul(po[:, 0, :], lhsT=wo_sb[:, 1, dot * P:(dot + 1) * P],
                                         rhs=ao23[:, ch * CHW:(ch + 1) * CHW], start=False, stop=True)
                        so = outp.tile([P, CHW], f32, tag="so")
                        if dot % 2 == 0:
                            nc.scalar.copy(out=so[:], in_=po[:, 0, :])
                        else:
                            nc.vector.tensor_copy(so[:], po[:, 0, :])
                        nc.sync.dma_start(out=out_d[b, dot, :, ch, :], in_=so[:])

                prev_ch = None
                for ch in (3, 2, 1, 0):
                    nks = 4 * (ch + 1)
                    for pair, (qt, ao) in enumerate(((qt01, ao01), (qt23, ao23))):
                        ppr = []
                        for kst in range(nks):
                            psc = ps_sc.tile([P, 2, CHW], f32, tag="sc")
                            nc.tensor.matmul(
                                psc[:, 0, :], lhsT=ktd[0:64, kst * P:(kst + 1) * P],
                                rhs=qt[0:64, ch * CHW:(ch + 1) * CHW], start=True, stop=True)
                            nc.tensor.matmul(
                                psc[:, 1, :], lhsT=ktd[64:128, kst * P:(kst + 1) * P],
                                rhs=qt[64:128, ch * CHW:(ch + 1) * CHW], start=True, stop=True)
                            pt = pp.tile([P, 2, CHW], mybir.dt.bfloat16, tag="p")
                            nc.scalar.activation(pt[:], psc[:], mybir.ActivationFunctionType.Exp, scale=0.125)
                            o = kst - 4 * ch
                            if o >= 0:
                                nc.gpsimd.affine_select(
                                    out=pt[:], in_=pt[:],
                                    compare_op=mybir.AluOpType.is_ge,
                                    fill=0.0, base=-P * o, channel_multiplier=-1,
                                    pattern=[[0, 2], [1, CHW]],
                                )
                            ppr.append(pt)
                        u0 = ps_u.tile([P, CHW], f32, tag="u")
                        u1 = ps_u.tile([P, CHW], f32, tag="u")
                        for kst in range(nks):
                            nc.tensor.matmul(u0[0:65, :], lhsT=v1[:, kst, :], rhs=ppr[kst][:, 0, :],
                                             start=(kst == 0), stop=(kst == nks - 1))
                            nc.tensor.matmul(u1[0:65, :], lhsT=v1[:, kst, :], rhs=ppr[kst][:, 1, :],
                                             start=(kst == 0), stop=(kst == nks - 1))
                        for u, basep in ((u0, 0), (u1, 64)):
                            dn = norm.tile([1, CHW], f32, tag="dn")
                            nc.vector.reciprocal(dn[:], u[64:65, :])
                            bcs = normu.tile([64, CHW], f32, tag="bcs")
                            nc.gpsimd.partition_broadcast(bcs[:], dn[:])
                            nc.vector.tensor_mul(
                                ao[basep:basep + 64, ch * CHW:(ch + 1) * CHW],
                                u[0:64, :], bcs[:])

                    # O-projection of the previous chunk (deps long satisfied)
                    if prev_ch is not None:
                        oproj_chunk(prev_ch)
                    prev_ch = ch
                oproj_chunk(prev_ch)

    nc.compile()
    return nc


def get_nc():
    global _nc_cache
    if _nc_cache is None:
        _nc_cache = build_nc()
    return _nc_cache


def prep_inputs(x, freqs_cos, freqs_sin, wq, wk, wv, wo):
    """Host-side layout prep. Returns list of per-core input dicts."""
    bf = ml_dtypes.bfloat16
    x = np.asarray(x, dtype=np.float32)
    # xh[b, st, p, ct, sl] = x[b, st*128+sl, ct*128+p]
    xh = np.ascontiguousarray(
        x.reshape(B, ST, P, CT, P).transpose(0, 1, 4, 3, 2).astype(bf))
    # fc[p, st, j] = freqs_cos[st*128+p, j]
    fc = np.ascontiguousarray(
        np.asarray(freqs_cos, np.float32).reshape(ST, P, 32).transpose(1, 0, 2))
    fs = np.ascontiguousarray(
        np.asarray(freqs_sin, np.float32).reshape(ST, P, 32).transpose(1, 0, 2))
    perm = np.concatenate([np.arange(0, HD, 2), np.arange(1, HD, 2)])
    in_maps = []
    for c in range(NCORE):
        q_rows = np.asarray(wq, np.float32)[c * HPC * HD:(c + 1) * HPC * HD]
        q_rows = q_rows.reshape(HPC, HD, DIM)[:, perm, :].reshape(HPC * HD, DIM)
        k_rows = np.asarray(wk, np.float32)[c * HD:(c + 1) * HD][perm]
        v_rows = np.asarray(wv, np.float32)[c * HD:(c + 1) * HD]
        wcat = np.concatenate([q_rows, k_rows, v_rows], axis=0)  # [384, DIM]
        w_h = np.ascontiguousarray(wcat.T.reshape(CT, P, QKV).transpose(1, 0, 2).astype(bf))
        wo_cols = np.asarray(wo, np.float32)[:, c * HPC * HD:(c + 1) * HPC * HD]  # [DIM, 256]
        wo_h = np.ascontiguousarray(wo_cols.T.reshape(2, P, DIM).transpose(1, 0, 2).astype(bf))
        in_maps.append({"xt": xh, "wqkv": w_h, "wo": wo_h, "fcos": fc, "fsin": fs})
    return in_maps


def combine_outputs(results):
    """Sum per-core partial out^T and return [B, S, DIM] float32."""
    acc = np.zeros((B, ST, P, NCH, CHW), np.float64)
    for r in results:
        acc += r["out"].astype(np.float64)
    # out[b, ch*512+sl, dot*128+p] = acc[b, dot, p, ch, sl]
    return np.ascontiguousarray(
        acc.transpose(0, 3, 4, 1, 2).reshape(B, S, DIM).astype(np.float32))


def kernel(x, freqs_cos, freqs_sin, wq, wk, wv, wo):
    from concourse.bass_utils import run_bass_kernel_spmd

    nc = get_nc()
    in_maps = prep_inputs(x, freqs_cos, freqs_sin, wq, wk, wv, wo)
    res = run_bass_kernel_spmd(nc, in_maps, core_ids=list(range(NCORE)))
    return combine_outputs(res.results)


# revision 25
# speedup vs baseline: 1.4421x; 1.0108x over previous
"""GQA causal attention with rope, 8-way head-tensor-parallel on one TRN2 chip.

Strategy (per core c of 8): q-heads 4c..4c+3, kv-head c.
  - Host pre-transposes/pre-tiles x, permutes wq/wk rows for cheap on-device rope,
    duplicates the k rows (k2) so k^T lands duplicated across both partition
    halves for row-tiled score matmuls, casts weights+x to bf16.
  - Device: QKV projection (TensorE, bf16) -> rope (VectorE, free-dim pair blocks)
    -> DMA-transposes to q^T/k^T layout -> scores S^T = K Q^T (row-tiled 64x128
    matmul pairs, 2 heads concurrently, into one 2-bank PSUM tile) -> single
    paired exp on ScalarE (no max subtraction; scores ~ N(0,1)) -> causal mask
    via gpsimd affine_select on diagonal tiles -> P^T V with a fused ones-column
    producing softmax denominators -> normalize via partition_broadcast +
    ln/exp reciprocal -> output projection interleaved per qs-chunk.
  - Each core DMAs out its partial out^T [B, D, S]; host sums partials over
    cores and transposes back.
"""
import sys
for _p in ("/opt/trn_rl_repo",):
    if _p not in sys.path:
        sys.path.insert(0, _p)

import numpy as np
import ml_dtypes

B, S, DIM = 2, 2048, 2048
NH, NKV, HD = 32, 8, 64
P = 128
ST = S // P          # 16 s-tiles
CT = DIM // P        # 16 contraction tiles
NCORE = 8
HPC = NH // NCORE    # 4 q heads per core
QKV = 384            # 4*64 q + 64 k + 64 v columns
NROPE = 320          # rope'd columns (q + k)
NCH = 4              # qs chunks of 512
CHW = 512

_nc_cache = None


def build_nc():
    import concourse.bass as bass
    import concourse.mybir as mybir
    import concourse.tile as tile
    from concourse import bacc
    from concourse.masks import make_identity

    f32 = mybir.dt.float32
    bf16 = mybir.dt.bfloat16

    nc = bacc.Bacc("TRN2", target_bir_lowering=False)
    xt_d = nc.declare_dram_parameter("xt", [B, ST, P, CT, P], bf16, isOutput=False)
    w_d = nc.declare_dram_parameter("wqkv", [P, CT, QKV], bf16, isOutput=False)
    wo_d = nc.declare_dram_parameter("wo", [P, 2, DIM], bf16, isOutput=False)
    fc_d = nc.declare_dram_parameter("fcos", [P, ST, 32], f32, isOutput=False)
    fs_d = nc.declare_dram_parameter("fsin", [P, ST, 32], f32, isOutput=False)
    out_d = nc.declare_dram_parameter("out", [B, ST, P, NCH, CHW], f32, isOutput=True)

    AP = bass.AP

    def blocks(t, col0, nblk, bstride=64):
        """AP over `nblk` 32-wide col blocks of 2D tile t starting at col0, stride bstride."""
        a = t if isinstance(t, AP) else t[:]
        return AP(tensor=a.tensor, offset=a.offset + col0, ap=[a.ap[0], [bstride, nblk], [1, 32]])

    def bcast32(a, nblk):
        """Broadcast a [128, 32] AP across nblk col blocks."""
        return AP(tensor=a.tensor, offset=a.offset, ap=[a.ap[0], [0, nblk], [1, 32]])

    with tile.TileContext(nc) as tc:
        with (
            tc.tile_pool(name="const", bufs=1) as cst,
            tc.tile_pool(name="work", bufs=3) as work,
            tc.tile_pool(name="perb", bufs=2) as perb,
            tc.tile_pool(name="pp", bufs=18) as pp,
            tc.tile_pool(name="norm", bufs=3) as norm,
            tc.tile_pool(name="normu", bufs=6) as normu,
            tc.tile_pool(name="outp", bufs=4) as outp,
            tc.tile_pool(name="ps_sc", bufs=3, space="PSUM") as ps_sc,
            tc.tile_pool(name="ps_u", bufs=2, space="PSUM") as ps_u,
        ):
            w_sb = cst.tile([P, CT, QKV], bf16, tag="w")
            nc.sync.dma_start(out=w_sb[:], in_=w_d[:])
            wo_sb = cst.tile([P, 2, DIM], bf16, tag="wo")
            nc.sync.dma_start(out=wo_sb[:], in_=wo_d[:])
            fc_sb = cst.tile([P, ST, 32], f32, tag="fc")
            nc.sync.dma_start(out=fc_sb[:], in_=fc_d[:])
            fs_sb = cst.tile([P, ST, 32], f32, tag="fs")
            nc.sync.dma_start(out=fs_sb[:], in_=fs_d[:])
            ident = cst.tile([P, P], bf16, tag="id")
            make_identity(nc, ident)

            # static causal mask tiles for the all-diagonal (ch==0) strips: DVE-applied
            cmask = cst.tile([P, 4, CHW], bf16, tag="cmask")
            nc.gpsimd.memset(cmask[:], 1.0)
            # keep where qs - ks - 128*o >= 0 (free dims: [o 4][qs 512], partition = ks)
            nc.gpsimd.affine_select(
                out=cmask[:], in_=cmask[:], compare_op=mybir.AluOpType.is_ge,
                fill=0.0, base=0, channel_multiplier=-1, pattern=[[-P, 4], [1, CHW]])

            tiles = {}
            for b in range(B):
                qt01 = perb.tile([P, S], bf16, tag="qt01")
                qt23 = perb.tile([P, S], bf16, tag="qt23")
                ktd = perb.tile([P, S], bf16, tag="ktd")
                v1 = perb.tile([P, ST, 65], bf16, tag="v1")
                ao01 = perb.tile([P, S], bf16, tag="ao01")
                ao23 = perb.tile([P, S], bf16, tag="ao23")
                tiles[b] = (qt01, qt23, ktd, v1, ao01, ao23)
                nc.vector.memset(v1[:], 1.0)  # ones col; data cols overwritten below

                # ---- Phase A: QKV projection + rope + transposes ----
                for st in range(ST):
                    xt = work.tile([P, CT, P], bf16, tag="xt")
                    nc.sync.dma_start(out=xt[:], in_=xt_d[b, st])
                    pmm = ps_sc.tile([P, 2, CHW], f32, tag="sc")
                    for ct in range(CT):
                        nc.tensor.matmul(
                            pmm[:, 0, 0:QKV], lhsT=xt[:, ct, :], rhs=w_sb[:, ct, :],
                            start=(ct == 0), stop=(ct == CT - 1),
                        )
                    pm = pmm[:, 0, 0:QKV]
                    cos_st = fc_sb[:, st, :]
                    sin_st = fs_sb[:, st, :]
                    tA = work.tile([P, NROPE], f32, tag="tA")
                    tB = work.tile([P, NROPE], f32, tag="tB")
                    # tA = pm * cos on all 12 rope blocks (q0..q3,k,k2) x (t0,t1)
                    nc.vector.tensor_mul(blocks(tA, 0, 10, 32), blocks(pm, 0, 10, 32), bcast32(cos_st, 10))
                    # tB[t0 blocks] = pm[t1 blocks] * sin ; tB[t1] = pm[t0] * sin
                    nc.vector.tensor_mul(blocks(tB, 0, 5), blocks(pm, 32, 5), bcast32(sin_st, 5))
                    nc.vector.tensor_mul(blocks(tB, 32, 5), blocks(pm, 0, 5), bcast32(sin_st, 5))
                    qk = work.tile([P, NROPE], bf16, tag="qk")
                    nc.vector.tensor_sub(blocks(qk, 0, 5), blocks(tA, 0, 5), blocks(tB, 0, 5))
                    nc.vector.tensor_add(blocks(qk, 32, 5), blocks(tA, 32, 5), blocks(tB, 32, 5))
                    nc.vector.tensor_copy(v1[:, st, 0:64], pm[:, NROPE:QKV])
                    # PE transposes: q01, q23 [128,128]; k [128,64] dup'd to both halves
                    for j, dest in ((0, qt01), (1, qt23)):
                        ptr = ps_u.tile([P, CHW], mybir.dt.bfloat16, tag="u")
                        nc.tensor.transpose(ptr[:, 0:P], qk[:, j * P:(j + 1) * P], ident[:])
                        if j == 0:
                            nc.scalar.copy(out=dest[:, st * P:(st + 1) * P], in_=ptr[:, 0:P])
                        else:
                            nc.vector.tensor_copy(dest[:, st * P:(st + 1) * P], ptr[:, 0:P])
                    ptk = ps_u.tile([P, CHW], mybir.dt.bfloat16, tag="u")
                    nc.tensor.transpose(ptk[0:64, 0:P], qk[:, 256:320], ident[:])
                    nc.scalar.copy(out=ktd[0:64, st * P:(st + 1) * P], in_=ptk[0:64, 0:P])
                    nc.vector.tensor_copy(ktd[64:128, st * P:(st + 1) * P], ptk[0:64, 0:P])

            # ---- Phases B+C: both batches' strips interleaved, O-proj one chunk behind ----
            def oproj_chunk(b, ch):
                _, _, _, _, ao01, ao23 = tiles[b]
                for dot in range(ST):
                    po = ps_sc.tile([P, 2, CHW], f32, tag="sc")
                    nc.tensor.matmul(po[:, 0, :], lhsT=wo_sb[:, 0, dot * P:(dot + 1) * P],
                                     rhs=ao01[:, ch * CHW:(ch + 1) * CHW], start=True, stop=False)
                    nc.tensor.matmul(po[:, 0, :], lhsT=wo_sb[:, 1, dot * P:(dot + 1) * P],
                                     rhs=ao23[:, ch * CHW:(ch + 1) * CHW], start=False, stop=True)
                    so = outp.tile([P, CHW], f32, tag="so")
                    if dot % 2 == 0:
                        nc.scalar.copy(out=so[:], in_=po[:, 0, :])
                    else:
                        nc.vector.tensor_copy(so[:], po[:, 0, :])
                    nc.sync.dma_start(out=out_d[b, dot, :, ch, :], in_=so[:])

            def strip(b, pair, ch):
                qt01, qt23, ktd, v1, ao01, ao23 = tiles[b]
                qt, ao = (qt01, ao01) if pair == 0 else (qt23, ao23)
                nks = 4 * (ch + 1)
                ppr = []
                for kst in range(nks):
                    psc = ps_sc.tile([P, 2, CHW], f32, tag="sc")
                    nc.tensor.matmul(
                        psc[:, 0, :], lhsT=ktd[0:64, kst * P:(kst + 1) * P],
                        rhs=qt[0:64, ch * CHW:(ch + 1) * CHW], start=True, stop=True)
                    nc.tensor.matmul(
                        psc[:, 1, :], lhsT=ktd[64:128, kst * P:(kst + 1) * P],
                        rhs=qt[64:128, ch * CHW:(ch + 1) * CHW], start=True, stop=True)
                    pt = pp.tile([P, 2, CHW], mybir.dt.bfloat16, tag="p")
                    nc.scalar.activation(pt[:], psc[:], mybir.ActivationFunctionType.Exp, scale=0.125)
                    o = kst - 4 * ch
                    if o >= 0:
                        if ch == 0:
                            # all-diagonal strip: DVE mask (keeps PV off the gpsimd queue)
                            msk = bass.AP(tensor=cmask[:].tensor,
                                          offset=cmask[:].offset + o * CHW,
                                          ap=[cmask[:].ap[0], [0, 2], [1, CHW]])
                            nc.vector.tensor_mul(pt[:], pt[:], msk)
                        else:
                            nc.gpsimd.affine_select(
                                out=pt[:], in_=pt[:],
                                compare_op=mybir.AluOpType.is_ge,
                                fill=0.0, base=-P * o, channel_multiplier=-1,
                                pattern=[[0, 2], [1, CHW]],
                            )
                    ppr.append(pt)
                u0 = ps_u.tile([P, CHW], f32, tag="u")
                u1 = ps_u.tile([P, CHW], f32, tag="u")
                for kst in range(nks):
                    nc.tensor.matmul(u0[0:65, :], lhsT=v1[:, kst, :], rhs=ppr[kst][:, 0, :],
                                     start=(kst == 0), stop=(kst == nks - 1))
                    nc.tensor.matmul(u1[0:65, :], lhsT=v1[:, kst, :], rhs=ppr[kst][:, 1, :],
                                     start=(kst == 0), stop=(kst == nks - 1))
                for u, basep in ((u0, 0), (u1, 64)):
                    dn = norm.tile([1, CHW], f32, tag="dn")
                    nc.vector.reciprocal(dn[:], u[64:65, :])
                    bcs = normu.tile([64, CHW], f32, tag="bcs")
                    nc.gpsimd.partition_broadcast(bcs[:], dn[:])
                    nc.vector.tensor_mul(
                        ao[basep:basep + 64, ch * CHW:(ch + 1) * CHW],
                        u[0:64, :], bcs[:])

            prev = None
            for ch in (3, 2, 1, 0):
                for b in range(B):
                    strip(b, 0, ch)
                    strip(b, 1, ch)
                    if prev is not None:
                        oproj_chunk(*prev)
                    prev = (b, ch)
            oproj_chunk(*prev)

    nc.compile()
    return nc


def get_nc():
    global _nc_cache
    if _nc_cache is None:
        _nc_cache = build_nc()
    return _nc_cache


def prep_inputs(x, freqs_cos, freqs_sin, wq, wk, wv, wo):
    """Host-side layout prep. Returns list of per-core input dicts."""
    bf = ml_dtypes.bfloat16
    x = np.asarray(x, dtype=np.float32)
    # xh[b, st, p, ct, sl] = x[b, st*128+sl, ct*128+p]
    xh = np.ascontiguousarray(
        x.reshape(B, ST, P, CT, P).transpose(0, 1, 4, 3, 2).astype(bf))
    # fc[p, st, j] = freqs_cos[st*128+p, j]
    fc = np.ascontiguousarray(
        np.asarray(freqs_cos, np.float32).reshape(ST, P, 32).transpose(1, 0, 2))
    fs = np.ascontiguousarray(
        np.asarray(freqs_sin, np.float32).reshape(ST, P, 32).transpose(1, 0, 2))
    perm = np.concatenate([np.arange(0, HD, 2), np.arange(1, HD, 2)])
    in_maps = []
    for c in range(NCORE):
        q_rows = np.asarray(wq, np.float32)[c * HPC * HD:(c + 1) * HPC * HD]
        q_rows = q_rows.reshape(HPC, HD, DIM)[:, perm, :].reshape(HPC * HD, DIM)
        k_rows = np.asarray(wk, np.float32)[c * HD:(c + 1) * HD][perm]
        v_rows = np.asarray(wv, np.float32)[c * HD:(c + 1) * HD]
        wcat = np.concatenate([q_rows, k_rows, v_rows], axis=0)  # [384, DIM]
        w_h = np.ascontiguousarray(wcat.T.reshape(CT, P, QKV).transpose(1, 0, 2).astype(bf))
        wo_cols = np.asarray(wo, np.float32)[:, c * HPC * HD:(c + 1) * HPC * HD]  # [DIM, 256]
        wo_h = np.ascontiguousarray(wo_cols.T.reshape(2, P, DIM).transpose(1, 0, 2).astype(bf))
        in_maps.append({"xt": xh, "wqkv": w_h, "wo": wo_h, "fcos": fc, "fsin": fs})
    return in_maps


def combine_outputs(results):
    """Sum per-core partial out^T and return [B, S, DIM] float32."""
    acc = np.zeros((B, ST, P, NCH, CHW), np.float64)
    for r in results:
        acc += r["out"].astype(np.float64)
    # out[b, ch*512+sl, dot*128+p] = acc[b, dot, p, ch, sl]
    return np.ascontiguousarray(
        acc.transpose(0, 3, 4, 1, 2).reshape(B, S, DIM).astype(np.float32))


def kernel(x, freqs_cos, freqs_sin, wq, wk, wv, wo):
    from concourse.bass_utils import run_bass_kernel_spmd

    nc = get_nc()
    in_maps = prep_inputs(x, freqs_cos, freqs_sin, wq, wk, wv, wo)
    res = run_bass_kernel_spmd(nc, in_maps, core_ids=list(range(NCORE)))
    return combine_outputs(res.results)
